# revision 3
# baseline (speedup 1.0000x reference)
"""CustomLSTMCell kernel for Trainium2, data-parallel over batch on 8 cores.

Math (per token, elementwise over dff except the GEMM):
    gates = [Hi|Zi] @ [Wh;Wz] + bias         # [tok, 4*dff], gate order I|F|O|Z
    A = F~ + Mi;  M_t = A - min(A - I~, 0)   # = max(A, I~)
    F_t = exp(min(A - I~, 0));  I_t = exp(min(I~ - A, 0))
    O_t = 0.5*(1 + tanh(O~/2));  Z_t = tanh(Z~)
    N_t = F_t*Ni + I_t
    C_t = (Ci*F_t + Z_t*I_t)*m + (1-m)*Ci
    H_t = O_t*(C_t/N_t)*m + (1-m)*Hi

This workload is HBM-bandwidth-bound on the shared 8-core device, so the
design minimizes bytes (50 MiB/core/exec vs 86 for an fp32-elementwise
version) while staying on the fp16 PE fast path (fp8 DoubleRow measured
slower per MAC-column than modeled on this stack):
- fp16 GEMM, tokens on partitions, activations stationary, weights moving,
  fp32 PSUM accumulate; no PE bias seeds (O/Z biases added by DVE at PSUM
  readout from partition-broadcast tiles; F bias folded into Mi on the host;
  I bias cancels in F_t/I_t and is added to M_t on the host after gather).
- fp16 elementwise in SBUF (DVE 2x/4x perf modes; N_t reciprocal in fp32).
- Packed DMA: the 4 elementwise inputs and 4 outputs travel as single
  [tok, chunk, 4, 512] tensors (4 KiB contiguous runs, 1 descriptor each
  per tile instead of 4); outputs are fp16, upcast on the host.
- Full next-chunk weight double-buffering (wpool 2*KT) to avoid a PE bubble
  at the column-chunk boundary.
Engine split keeps DVE/Act/GPSIMD each under the 10.2 us/tile PE time.
"""

import os
import numpy as np

import concourse.bass as bass
import concourse.tile as tile
import concourse.bass_utils as bass_utils
from concourse import bacc, mybir
from concourse.bass import ts, ds

# --- Ldweights dedup -------------------------------------------------------
# tile_legalize splits every InstMatmult into InstLdweights + InstMatmult
# (ldweights=False) with no dedup, so 4 gate matmuls sharing one stationary
# tile reload the PE array 4x. On HW each redundant load serializes with the
# matmul stream (~53ns w/ FWL), costing ~55us/exec. Drop an InstLdweights
# when the previous PE-stream weight load has the identical signature; deps
# of the duplicates are identical to the kept load (same stationary tile
# version), and nothing depends on an InstLdweights (PE executes in order),
# so sem assignment downstream (TileClockWait runs after this) stays sound.
_ldw_patched = False


def _ldw_sig(inst):
    return (str(inst.ins), str(inst.perf_mode), str(inst.is_transpose),
            str(inst.tile_position))


def _install_ldw_dedup():
    global _ldw_patched
    if _ldw_patched:
        return
    _ldw_patched = True
    orig = tile.tile_legalize

    def deduped(ordered, nc):
        res = orig(ordered, nc)
        dropped = 0
        for bb in list(res.keys()):
            insts = res[bb]
            out = []
            last_sig = None
            for inst in insts:
                tn = type(inst).__name__
                if tn == "InstLdweights":
                    sig = _ldw_sig(inst)
                    if sig == last_sig:
                        dropped += 1
                        continue
                    last_sig = sig
                elif inst.engine == mybir.EngineType.PE:
                    if tn == "InstMatmult":
                        if inst.is_transpose:
                            last_sig = None
                    elif not inst.is_sequencer_only:
                        last_sig = None
                out.append(inst)
            res[bb] = out
        if dropped:
            print(f"[kernel] ldweights dedup: dropped {dropped}")
        return res

    tile.tile_legalize = deduped

B, P, D, DFF = 256, 64, 512, 1024
NCORES = 8
BL = B // NCORES
TOK = BL * P              # 2048
NT = TOK // 128           # 16
KH = DFF // 128           # 8
KZ = D // 128             # 4
KT = KH + KZ              # 12
CH = 2
CW = 512

F32 = mybir.dt.float32
F16 = mybir.dt.float16
AF = mybir.ActivationFunctionType
OP = mybir.AluOpType

_CACHE = {}


def _build(repeat: int = 1):
    if repeat in _CACHE:
        return _CACHE[repeat]

    _install_ldw_dedup()
    nc = bacc.Bacc("TRN2", target_bir_lowering=False, debug=False,
                   num_devices=NCORES)

    xT = nc.dram_tensor("xT", [KT, 128, TOK], F16, kind="ExternalInput").ap()
    w = nc.dram_tensor("w", [CH, KT, 128, 4, CW], F16, kind="ExternalInput").ap()
    boz = nc.dram_tensor("boz", [2, CH, CW], F32, kind="ExternalInput").ap()
    # packed elementwise inputs: [tok, chunk, (mi|ci|ni|ho), cw] - 4KiB runs
    elin = nc.dram_tensor("elin", [TOK, CH, 4, CW], F16,
                          kind="ExternalInput").ap()
    mpk = nc.dram_tensor("mpk", [NT, 128, 3], F32, kind="ExternalInput").ap()

    # packed outputs: [tok, chunk, (mt|nt|ct|ht), cw]
    out = nc.dram_tensor("out", [TOK, CH, 4, CW], F16,
                         kind="ExternalOutput").ap()

    with tile.TileContext(nc) as tc:
        with (
            tc.tile_pool(name="singles", bufs=1) as singles,
            tc.tile_pool(name="wpool", bufs=2 * KT) as wpool,
            tc.tile_pool(name="inpool", bufs=3) as inpool,
            tc.tile_pool(name="tmpA", bufs=1) as tmpA,
            tc.tile_pool(name="tmpB", bufs=2) as tmpB,
            tc.tile_pool(name="outp", bufs=3) as outp,
            tc.tile_pool(name="ps", bufs=8, space="PSUM") as pspool,
        ):
            xk_sb = []
            for k in range(KT):
                xk = singles.tile([128, TOK], F16, name=f"xT{k}")
                nc.sync.dma_start(out=xk, in_=xT[k])
                xk_sb.append(xk)
            mpk_sb = singles.tile([128, NT, 3], F32)
            nc.sync.dma_start(out=mpk_sb, in_=mpk.rearrange("t p c -> p t c"))
            # partition-broadcast O/Z bias tiles [128, CH, CW]
            bb_sb = singles.tile([128, 2, CH, CW], F32)
            for gi in range(2):
                for cj in range(CH):
                    bsl = boz[gi, cj]
                    bcast = bass.AP(tensor=bsl.tensor, offset=bsl.offset,
                                    ap=[[0, 128]] + list(bsl.ap))
                    nc.gpsimd.dma_start(out=bb_sb[:, gi, cj], in_=bcast)

            for _ in range(repeat):
                for c in range(CH):
                    wk = []
                    for k in range(KT):
                        wt = wpool.tile([128, 4, CW], F16, tag="wk")
                        nc.sync.dma_start(out=wt, in_=w[c, k])
                        wk.append(wt)
                    for t in range(NT):
                        rows = ts(t, 128)
                        cols = ds(c * CW, CW)
                        ein = inpool.tile([128, 4, CW], F16, tag="ein")
                        nc.sync.dma_start(out=ein, in_=elin[rows, c])
                        mi_t, ci_t, ni_t, ho_t = (ein[:, 0], ein[:, 1],
                                                  ein[:, 2], ein[:, 3])
                        m_ap = mpk_sb[:, t, 0:1]
                        om_ap = mpk_sb[:, t, 1:2]
                        hm_ap = mpk_sb[:, t, 2:3]

                        ps = [pspool.tile([128, CW], F32, tag="ps",
                                          name=f"ps{g}") for g in range(4)]
                        for k in range(KT):
                            lhsT = xk_sb[k][:, rows]
                            for g in range(4):
                                nc.tensor.matmul(ps[g], lhsT, wk[k][:, g],
                                                 start=(k == 0),
                                                 stop=(k == KT - 1))

                        psI, psF, psO, psZ = ps
                        tmpI = tmpB.tile([128, CW], F16, tag="tmpI")
                        nc.scalar.activation(tmpI, psI, AF.Copy)
                        A = tmpA.tile([128, CW], F16, tag="A")
                        nc.vector.tensor_add(A, psF, mi_t)
                        tO = tmpA.tile([128, CW], F32, tag="tO")
                        nc.vector.tensor_add(tO, psO, bb_sb[:, 0, c])
                        th = tmpB.tile([128, CW], F16, tag="th")
                        nc.scalar.activation(th, tO, AF.Tanh, scale=0.5)
                        tZ = tmpA.tile([128, CW], F32, tag="tZ")
                        nc.vector.tensor_add(tZ, psZ, bb_sb[:, 1, c])
                        Zt = tmpB.tile([128, CW], F16, tag="Zt")
                        nc.scalar.activation(Zt, tZ, AF.Tanh)

                        Dd = tmpA.tile([128, CW], F16, tag="Dd")
                        nc.vector.tensor_sub(Dd, A, tmpI)
                        p_ = tmpA.tile([128, CW], F16, tag="p")
                        nc.vector.tensor_scalar_min(p_, Dd, 0.0)
                        pn = tmpA.tile([128, CW], F16, tag="pn")
                        nc.vector.tensor_scalar(pn, Dd, -1.0, 0.0, OP.mult,
                                                OP.min)
                        eout = outp.tile([128, 4, CW], F16, tag="eout")
                        Mt, Nt, Ct, Ht = (eout[:, 0], eout[:, 1],
                                          eout[:, 2], eout[:, 3])
                        nc.vector.tensor_sub(Mt, A, p_)
                        Ft = tmpB.tile([128, CW], F16, tag="Ft")
                        nc.scalar.activation(Ft, p_, AF.Exp)
                        It = tmpB.tile([128, CW], F16, tag="It")
                        nc.scalar.activation(It, pn, AF.Exp)

                        FN = tmpA.tile([128, CW], F16, tag="FN")
                        nc.gpsimd.tensor_mul(FN, Ft, ni_t)
                        NtF = tmpA.tile([128, CW], F32, tag="NtF")
                        nc.gpsimd.tensor_add(NtF, FN, It)
                        nc.scalar.activation(Nt, NtF, AF.Copy)
                        rec = tmpB.tile([128, CW], F32, tag="rec")
                        nc.vector.reciprocal_approx_fast(rec, NtF)

                        mF = tmpA.tile([128, CW], F16, tag="mF")
                        nc.vector.tensor_scalar(mF, Ft, m_ap, om_ap, OP.mult,
                                                OP.add)
                        p1 = tmpA.tile([128, CW], F16, tag="p1")
                        nc.gpsimd.tensor_mul(p1, ci_t, mF)
                        t2 = tmpA.tile([128, CW], F16, tag="t2")
                        nc.gpsimd.tensor_mul(t2, Zt, It)
                        nc.vector.scalar_tensor_tensor(Ct, t2, m_ap, p1,
                                                       OP.mult, OP.add)

                        thp = tmpA.tile([128, CW], F16, tag="thp")
                        nc.vector.tensor_scalar(thp, th, hm_ap, hm_ap,
                                                OP.mult, OP.add)
                        x1 = tmpA.tile([128, CW], F16, tag="x1")
                        nc.vector.tensor_mul(x1, Ct, rec)
                        x2 = tmpA.tile([128, CW], F16, tag="x2")
                        nc.vector.tensor_mul(x2, x1, thp)
                        nc.vector.tensor_add(Ht, x2, ho_t)

                        nc.sync.dma_start(out=out[rows, c], in_=eout)

    nc.compile()
    _CACHE[repeat] = nc
    return nc


def _prep_inputs(inputs):
    f32, f16 = np.float32, np.float16
    g = {k: np.asarray(v) for k, v in inputs.items()}

    Wh = np.concatenate([g['WI_w'], g['WF_w'], g['WO_w'], g['WZ_w']], axis=1)
    Wz = np.concatenate([g['RI_w'], g['RF_w'], g['RO_w'], g['RZ_w']], axis=1)
    bias = np.concatenate([g['WI_b'] + g['RI_b'], g['WF_b'] + g['RF_b'],
                           g['WO_b'] + g['RO_b'], g['WZ_b'] + g['RZ_b']])
    Wcat = np.vstack([Wh, Wz]).astype(f16)                   # [1536, 4096]
    w_l = np.ascontiguousarray(
        Wcat.reshape(KT, 128, 4, CH, CW).transpose(3, 0, 1, 2, 4))
    bI, bF, bO, bZ = bias.reshape(4, DFF).astype(f32)
    boz_l = np.ascontiguousarray(np.stack([bO, bZ]).reshape(2, CH, CW))
    mi_shift = (bF - bI)[None, :]

    in_maps = []
    for c in range(NCORES):
        sl = slice(c * BL, (c + 1) * BL)
        Hi_c = g['Hi'][sl].reshape(TOK, DFF)
        Zi_c = g['Zi'][sl].reshape(TOK, D)
        m_c = g['m'][sl].reshape(TOK, 1).astype(f32)
        X = np.concatenate([Hi_c, Zi_c], axis=1)
        xT = np.ascontiguousarray(X.T).astype(f16).reshape(KT, 128, TOK)
        mpk = np.concatenate([m_c, 1.0 - m_c, 0.5 * m_c],
                             axis=1).astype(f32).reshape(NT, 128, 3)
        elin = np.empty((TOK, CH, 4, CW), f16)
        for j, arr in enumerate([
                g['Mi'][sl].reshape(TOK, DFF) + mi_shift,
                g['Ci'][sl].reshape(TOK, DFF),
                g['Ni'][sl].reshape(TOK, DFF),
                (1.0 - m_c) * Hi_c]):
            elin[:, :, j, :] = arr.astype(f16).reshape(TOK, CH, CW)
        in_maps.append({
            "xT": xT,
            "w": w_l,
            "boz": boz_l,
            "elin": elin,
            "mpk": mpk,
        })
    return in_maps, bI


def _gather(results, bI):
    def cat(j):
        full = np.concatenate(
            [results[c]["out"][:, :, j, :].astype(np.float32)
             .reshape(BL, P, DFF) for c in range(NCORES)], axis=0)
        return np.ascontiguousarray(full, dtype=np.float32)
    mt = cat(0)
    mt += bI.reshape(1, 1, DFF)
    return cat(2), mt, cat(3), cat(1)


def kernel(**inputs):
    nc = _build(repeat=1)
    in_maps, bI = _prep_inputs(inputs)
    res = bass_utils.run_bass_kernel_spmd(nc, in_maps,
                                          core_ids=list(range(NCORES)))
    return _gather(res.results, bI)



# revision 6
# speedup vs baseline: 1.5148x; 1.5148x over previous
"""CustomLSTMCell kernel for Trainium2, data-parallel over batch on 8 cores.

Math (per token, elementwise over dff except the GEMM):
    gates = [Hi|Zi] @ [Wh;Wz] + bias         # [tok, 4*dff], gate order I|F|O|Z
    A = F~ + Mi;  M_t = A - min(A - I~, 0)   # = max(A, I~)
    F_t = exp(min(A - I~, 0));  I_t = exp(min(I~ - A, 0))
    O_t = 0.5*(1 + tanh(O~/2));  Z_t = tanh(Z~)
    N_t = F_t*Ni + I_t
    C_t = (Ci*F_t + Z_t*I_t)*m + (1-m)*Ci
    H_t = O_t*(C_t/N_t)*m + (1-m)*Hi

This workload is HBM-bandwidth-bound on the shared 8-core device, so the
design minimizes bytes (50 MiB/core/exec vs 86 for an fp32-elementwise
version) while staying on the fp16 PE fast path (fp8 DoubleRow measured
slower per MAC-column than modeled on this stack):
- fp16 GEMM, tokens on partitions, activations stationary, weights moving,
  fp32 PSUM accumulate; no PE bias seeds (O/Z biases added by DVE at PSUM
  readout from partition-broadcast tiles; F bias folded into Mi on the host;
  I bias cancels in F_t/I_t and is added to M_t on the host after gather).
- fp16 elementwise in SBUF (DVE 2x/4x perf modes; N_t reciprocal in fp32).
- Packed DMA: the 4 elementwise inputs and 4 outputs travel as single
  [tok, chunk, 4, 512] tensors (4 KiB contiguous runs, 1 descriptor each
  per tile instead of 4); outputs are fp16, upcast on the host.
- Full next-chunk weight double-buffering (wpool 2*KT) to avoid a PE bubble
  at the column-chunk boundary.
Engine split keeps DVE/Act/GPSIMD each under the 10.2 us/tile PE time.
"""

import os
import numpy as np

import concourse.bass as bass
import concourse.tile as tile
import concourse.bass_utils as bass_utils
from concourse import bacc, mybir
from concourse.bass import ts, ds

# --- Ldweights dedup -------------------------------------------------------
# tile_legalize splits every InstMatmult into InstLdweights + InstMatmult
# (ldweights=False) with no dedup, so 4 gate matmuls sharing one stationary
# tile reload the PE array 4x. On HW each redundant load serializes with the
# matmul stream (~53ns w/ FWL), costing ~55us/exec. Drop an InstLdweights
# when the previous PE-stream weight load has the identical signature; deps
# of the duplicates are identical to the kept load (same stationary tile
# version), and nothing depends on an InstLdweights (PE executes in order),
# so sem assignment downstream (TileClockWait runs after this) stays sound.
_ldw_patched = False


def _ldw_sig(inst):
    return (str(inst.ins), str(inst.perf_mode), str(inst.is_transpose),
            str(inst.tile_position))


def _install_ldw_dedup():
    global _ldw_patched
    if _ldw_patched:
        return
    _ldw_patched = True
    orig = tile.tile_legalize

    def deduped(ordered, nc):
        res = orig(ordered, nc)
        dropped = 0
        for bb in list(res.keys()):
            insts = res[bb]
            out = []
            last_sig = None
            for inst in insts:
                tn = type(inst).__name__
                if tn == "InstLdweights":
                    sig = _ldw_sig(inst)
                    if sig == last_sig:
                        dropped += 1
                        continue
                    last_sig = sig
                elif inst.engine == mybir.EngineType.PE:
                    if tn == "InstMatmult":
                        if inst.is_transpose:
                            last_sig = None
                    elif not inst.is_sequencer_only:
                        last_sig = None
                out.append(inst)
            res[bb] = out
        if dropped:
            print(f"[kernel] ldweights dedup: dropped {dropped}")
        return res

    tile.tile_legalize = deduped

B, P, D, DFF = 256, 64, 512, 1024
NCORES = 8
BL = B // NCORES
TOK = BL * P              # 2048
NT = TOK // 128           # 16
KH = DFF // 128           # 8
KZ = D // 128             # 4
KT = KH + KZ              # 12
CH = 2
CW = 512

F32 = mybir.dt.float32
F16 = mybir.dt.float16
AF = mybir.ActivationFunctionType
OP = mybir.AluOpType

_CACHE = {}


def _build(repeat: int = 1):
    if repeat in _CACHE:
        return _CACHE[repeat]

    # NOTE: _install_ldw_dedup() measured SLOWER on HW (636us vs 411us):
    # breaking the per-matmul LDW+MM pairing falls off the PE fast path.
    nc = bacc.Bacc("TRN2", target_bir_lowering=False, debug=False,
                   num_devices=NCORES)

    xT = nc.dram_tensor("xT", [KT, 128, TOK], F16, kind="ExternalInput").ap()
    w = nc.dram_tensor("w", [CH, KT, 128, 4, CW], F16, kind="ExternalInput").ap()
    boz = nc.dram_tensor("boz", [2, CH, CW], F16, kind="ExternalInput").ap()
    # packed elementwise inputs: [tok, chunk, (mi|ci|ni|ho), cw] - 4KiB runs
    elin = nc.dram_tensor("elin", [TOK, CH, 4, CW], F16,
                          kind="ExternalInput").ap()
    mpk = nc.dram_tensor("mpk", [NT, 128, 3], F32, kind="ExternalInput").ap()

    # packed outputs: [tok, chunk, (mt|nt|ct|ht), cw]
    out = nc.dram_tensor("out", [TOK, CH, 4, CW], F16,
                         kind="ExternalOutput").ap()

    with tile.TileContext(nc) as tc:
        with (
            tc.tile_pool(name="singles", bufs=1) as singles,
            tc.tile_pool(name="wpool", bufs=KT + 6) as wpool,
            tc.tile_pool(name="inpool", bufs=3) as inpool,
            tc.tile_pool(name="tmpA", bufs=2) as tmpA,
            tc.tile_pool(name="tmpB", bufs=2) as tmpB,
            tc.tile_pool(name="outp", bufs=3) as outp,
            tc.tile_pool(name="ps", bufs=8, space="PSUM") as pspool,
        ):
            # Interleave xT k-tiles with the first chunk's weight tiles so the
            # first matmul starts after ~1 MiB of DMA instead of after the
            # whole 6 MiB xT block (saves ~15us off the single-exec startup).
            xk_sb = []
            wk_first = []
            for k in range(KT):
                xk = singles.tile([128, TOK], F16, name=f"xT{k}")
                nc.sync.dma_start(out=xk, in_=xT[k])
                xk_sb.append(xk)
                wt = wpool.tile([128, 4, CW], F16, tag="wk")
                nc.sync.dma_start(out=wt, in_=w[0, k])
                wk_first.append(wt)
            # small control tensors go on the gpsimd queue: available almost
            # immediately, never queued behind the bulk preamble
            mpk_sb = singles.tile([128, NT, 3], F32)
            nc.gpsimd.dma_start(out=mpk_sb,
                                in_=mpk.rearrange("t p c -> p t c"))
            # partition-broadcast O/Z bias tiles [128, CH, CW]
            bb_sb = singles.tile([128, 2, CH, CW], F16)
            for gi in range(2):
                for cj in range(CH):
                    bsl = boz[gi, cj]
                    bcast = bass.AP(tensor=bsl.tensor, offset=bsl.offset,
                                    ap=[[0, 128]] + list(bsl.ap))
                    nc.gpsimd.dma_start(out=bb_sb[:, gi, cj], in_=bcast)

            first = True
            for _ in range(repeat):
                for c in range(CH):
                    if first:
                        wk = wk_first
                        first = False
                    else:
                        wk = []
                        for k in range(KT):
                            wt = wpool.tile([128, 4, CW], F16, tag="wk")
                            nc.sync.dma_start(out=wt, in_=w[c, k])
                            wk.append(wt)
                    for t in range(NT):
                        rows = ts(t, 128)
                        ein = inpool.tile([128, 4, CW], F16, tag="ein")
                        nc.gpsimd.dma_start(out=ein, in_=elin[rows, c])
                        mi_t, ci_t, ni_t, ho_t = (ein[:, 0], ein[:, 1],
                                                  ein[:, 2], ein[:, 3])
                        m_ap = mpk_sb[:, t, 0:1]
                        om_ap = mpk_sb[:, t, 1:2]
                        hm_ap = mpk_sb[:, t, 2:3]

                        ps = [pspool.tile([128, CW], F32, tag="ps",
                                          name=f"ps{g}") for g in range(4)]
                        for k in range(KT):
                            lhsT = xk_sb[k][:, rows]
                            for g in range(4):
                                nc.tensor.matmul(ps[g], lhsT, wk[k][:, g],
                                                 start=(k == 0),
                                                 stop=(k == KT - 1))

                        psI, psF, psO, psZ = ps
                        tmpI = tmpB.tile([128, CW], F16, tag="tmpI")
                        nc.scalar.activation(tmpI, psI, AF.Copy)
                        A = tmpA.tile([128, CW], F16, tag="A")
                        nc.vector.tensor_add(A, psF, mi_t)
                        tO = tmpA.tile([128, CW], F16, tag="tO")
                        nc.vector.tensor_add(tO, psO, bb_sb[:, 0, c])
                        th = tmpB.tile([128, CW], F16, tag="th")
                        nc.scalar.activation(th, tO, AF.Tanh, scale=0.5)
                        tZ = tmpA.tile([128, CW], F16, tag="tZ")
                        nc.vector.tensor_add(tZ, psZ, bb_sb[:, 1, c])
                        Zt = tmpB.tile([128, CW], F16, tag="Zt")
                        nc.scalar.activation(Zt, tZ, AF.Tanh)

                        Dd = tmpA.tile([128, CW], F16, tag="Dd")
                        nc.vector.tensor_sub(Dd, A, tmpI)
                        p_ = tmpA.tile([128, CW], F16, tag="p")
                        nc.vector.tensor_scalar_min(p_, Dd, 0.0)
                        pn = tmpA.tile([128, CW], F16, tag="pn")
                        nc.vector.tensor_scalar(pn, Dd, -1.0, 0.0, OP.mult,
                                                OP.min)
                        eout = outp.tile([128, 4, CW], F16, tag="eout")
                        Mt, Nt, Ct, Ht = (eout[:, 0], eout[:, 1],
                                          eout[:, 2], eout[:, 3])
                        nc.vector.tensor_sub(Mt, A, p_)
                        Ft = tmpB.tile([128, CW], F16, tag="Ft")
                        nc.scalar.activation(Ft, p_, AF.Exp)
                        It = tmpB.tile([128, CW], F16, tag="It")
                        nc.scalar.activation(It, pn, AF.Exp)

                        # N path all on DVE (GpSimd at 0.42 eff was 1.1us/op
                        # and 2 hops of the critical path); NtF stays f32 for
                        # reciprocal_approx_fast's bit-layout requirement
                        FN = tmpA.tile([128, CW], F16, tag="FN")
                        nc.vector.tensor_mul(FN, Ft, ni_t)
                        NtF = tmpA.tile([128, CW], F32, tag="NtF")
                        nc.vector.tensor_add(NtF, FN, It)
                        nc.scalar.activation(Nt, NtF, AF.Copy)
                        rec = tmpB.tile([128, CW], F32, tag="rec")
                        nc.vector.reciprocal_approx_fast(rec, NtF)

                        mF = tmpA.tile([128, CW], F16, tag="mF")
                        nc.vector.tensor_scalar(mF, Ft, m_ap, om_ap, OP.mult,
                                                OP.add)
                        p1 = tmpA.tile([128, CW], F16, tag="p1")
                        nc.vector.tensor_mul(p1, ci_t, mF)
                        t2 = tmpA.tile([128, CW], F16, tag="t2")
                        nc.vector.tensor_mul(t2, Zt, It)
                        nc.vector.scalar_tensor_tensor(Ct, t2, m_ap, p1,
                                                       OP.mult, OP.add)

                        thp = tmpA.tile([128, CW], F16, tag="thp")
                        nc.vector.tensor_scalar(thp, th, hm_ap, hm_ap,
                                                OP.mult, OP.add)
                        x1 = tmpA.tile([128, CW], F16, tag="x1")
                        nc.vector.tensor_mul(x1, Ct, rec)
                        x2 = tmpA.tile([128, CW], F16, tag="x2")
                        nc.vector.tensor_mul(x2, x1, thp)
                        nc.vector.tensor_add(Ht, x2, ho_t)

                        nc.sync.dma_start(out=out[rows, c], in_=eout)

    nc.compile()
    _CACHE[repeat] = nc
    return nc


def _prep_inputs(inputs):
    f32, f16 = np.float32, np.float16
    g = {k: np.asarray(v) for k, v in inputs.items()}

    Wh = np.concatenate([g['WI_w'], g['WF_w'], g['WO_w'], g['WZ_w']], axis=1)
    Wz = np.concatenate([g['RI_w'], g['RF_w'], g['RO_w'], g['RZ_w']], axis=1)
    bias = np.concatenate([g['WI_b'] + g['RI_b'], g['WF_b'] + g['RF_b'],
                           g['WO_b'] + g['RO_b'], g['WZ_b'] + g['RZ_b']])
    Wcat = np.vstack([Wh, Wz]).astype(f16)                   # [1536, 4096]
    w_l = np.ascontiguousarray(
        Wcat.reshape(KT, 128, 4, CH, CW).transpose(3, 0, 1, 2, 4))
    bI, bF, bO, bZ = bias.reshape(4, DFF).astype(f32)
    boz_l = np.ascontiguousarray(
        np.stack([bO, bZ]).reshape(2, CH, CW).astype(f16))
    mi_shift = (bF - bI)[None, :]

    in_maps = []
    for c in range(NCORES):
        sl = slice(c * BL, (c + 1) * BL)
        Hi_c = g['Hi'][sl].reshape(TOK, DFF)
        Zi_c = g['Zi'][sl].reshape(TOK, D)
        m_c = g['m'][sl].reshape(TOK, 1).astype(f32)
        X = np.concatenate([Hi_c, Zi_c], axis=1)
        xT = np.ascontiguousarray(X.T).astype(f16).reshape(KT, 128, TOK)
        mpk = np.concatenate([m_c, 1.0 - m_c, 0.5 * m_c],
                             axis=1).astype(f32).reshape(NT, 128, 3)
        elin = np.empty((TOK, CH, 4, CW), f16)
        for j, arr in enumerate([
                g['Mi'][sl].reshape(TOK, DFF) + mi_shift,
                g['Ci'][sl].reshape(TOK, DFF),
                g['Ni'][sl].reshape(TOK, DFF),
                (1.0 - m_c) * Hi_c]):
            elin[:, :, j, :] = arr.astype(f16).reshape(TOK, CH, CW)
        in_maps.append({
            "xT": xT,
            "w": w_l,
            "boz": boz_l,
            "elin": elin,
            "mpk": mpk,
        })
    return in_maps, bI


def _gather(results, bI):
    def cat(j):
        full = np.concatenate(
            [results[c]["out"][:, :, j, :].astype(np.float32)
             .reshape(BL, P, DFF) for c in range(NCORES)], axis=0)
        return np.ascontiguousarray(full, dtype=np.float32)
    mt = cat(0)
    mt += bI.reshape(1, 1, DFF)
    return cat(2), mt, cat(3), cat(1)


def kernel(**inputs):
    nc = _build(repeat=1)
    in_maps, bI = _prep_inputs(inputs)
    res = bass_utils.run_bass_kernel_spmd(nc, in_maps,
                                          core_ids=list(range(NCORES)))
    return _gather(res.results, bI)



# revision 8
# speedup vs baseline: 1.6617x; 1.0970x over previous
"""CustomLSTMCell kernel for Trainium2, data-parallel over batch on 8 cores.

Math (per token, elementwise over dff except the GEMM):
    gates = [Hi|Zi] @ [Wh;Wz] + bias         # [tok, 4*dff], gate order I|F|O|Z
    A = F~ + Mi;  M_t = A - min(A - I~, 0)   # = max(A, I~)
    F_t = exp(min(A - I~, 0));  I_t = exp(min(I~ - A, 0))
    O_t = 0.5*(1 + tanh(O~/2));  Z_t = tanh(Z~)
    N_t = F_t*Ni + I_t
    C_t = (Ci*F_t + Z_t*I_t)*m + (1-m)*Ci
    H_t = O_t*(C_t/N_t)*m + (1-m)*Hi

The GEMM is the roofline: 1536 fp16 matmuls x 512 moving cols = 332.6 us
of PE time per core-exec; everything else is organized to hide behind it:
- fp16 GEMM, tokens on partitions, activations stationary, weights moving,
  fp32 PSUM accumulate; no PE bias seeds (O/Z biases added by DVE at PSUM
  readout from partition-broadcast tiles; F bias folded into Mi on the host;
  I bias cancels in F_t/I_t and is added to M_t on the host after gather).
  fp8 (DoubleRow) fails the 2e-2 gate: e4m3 gives ~8e-2 offline.
- All weights (12 MiB) + xT (6 MiB) stay RESIDENT in SBUF: no per-iteration
  weight traffic, less HBM contention across the 8 cores. SBUF is ~98% full.
- Preamble interleaves xT k-tiles with chunk-0 weights so the first matmul
  issues after ~1 MiB; ein/mpk/bias DMAs ride the gpsimd queue so they are
  never stuck behind the 18 MiB bulk preamble on the sync queue.
- fp16 elementwise in SBUF (DVE 2x/4x perf modes; N_t reciprocal in fp32).
  Per-engine FIFO issue order follows operand availability; the N_t/rec
  path (longest) gets priority; FN/p1/t2 run on the otherwise-idle GpSimd
  (GpSimd for PSUM-adjacent ops measured 50us/exec slower - its 0.42-eff
  1.1us ops sat on the critical path).
- Outputs leave as two [128,2,512] fp16 DMAs (Mt|Nt early, Ct|Ht at chain
  end) to shorten the post-GEMM drain; host upcasts and adds bI to M_t.
- Ldweights dedup (sharing one stationary load across the 4 gate matmuls)
  measured SLOWER on HW (636us vs 411us): breaking the per-matmul LDW+MM
  pairing falls off the PE fast path. _install_ldw_dedup is kept, unused.
"""

import os
import numpy as np

import concourse.bass as bass
import concourse.tile as tile
import concourse.bass_utils as bass_utils
from concourse import bacc, mybir
from concourse.bass import ts, ds

# --- Ldweights dedup -------------------------------------------------------
# tile_legalize splits every InstMatmult into InstLdweights + InstMatmult
# (ldweights=False) with no dedup, so 4 gate matmuls sharing one stationary
# tile reload the PE array 4x. On HW each redundant load serializes with the
# matmul stream (~53ns w/ FWL), costing ~55us/exec. Drop an InstLdweights
# when the previous PE-stream weight load has the identical signature; deps
# of the duplicates are identical to the kept load (same stationary tile
# version), and nothing depends on an InstLdweights (PE executes in order),
# so sem assignment downstream (TileClockWait runs after this) stays sound.
_ldw_patched = False


def _ldw_sig(inst):
    return (str(inst.ins), str(inst.perf_mode), str(inst.is_transpose),
            str(inst.tile_position))


def _install_ldw_dedup():
    global _ldw_patched
    if _ldw_patched:
        return
    _ldw_patched = True
    orig = tile.tile_legalize

    def deduped(ordered, nc):
        res = orig(ordered, nc)
        dropped = 0
        for bb in list(res.keys()):
            insts = res[bb]
            out = []
            last_sig = None
            for inst in insts:
                tn = type(inst).__name__
                if tn == "InstLdweights":
                    sig = _ldw_sig(inst)
                    if sig == last_sig:
                        dropped += 1
                        continue
                    last_sig = sig
                elif inst.engine == mybir.EngineType.PE:
                    if tn == "InstMatmult":
                        if inst.is_transpose:
                            last_sig = None
                    elif not inst.is_sequencer_only:
                        last_sig = None
                out.append(inst)
            res[bb] = out
        if dropped:
            print(f"[kernel] ldweights dedup: dropped {dropped}")
        return res

    tile.tile_legalize = deduped

B, P, D, DFF = 256, 64, 512, 1024
NCORES = 8
BL = B // NCORES
TOK = BL * P              # 2048
NT = TOK // 128           # 16
KH = DFF // 128           # 8
KZ = D // 128             # 4
KT = KH + KZ              # 12
CH = 2
CW = 512

F32 = mybir.dt.float32
F16 = mybir.dt.float16
AF = mybir.ActivationFunctionType
OP = mybir.AluOpType

_CACHE = {}


def _build(repeat: int = 1):
    if repeat in _CACHE:
        return _CACHE[repeat]

    # NOTE: _install_ldw_dedup() measured SLOWER on HW (636us vs 411us):
    # breaking the per-matmul LDW+MM pairing falls off the PE fast path.
    nc = bacc.Bacc("TRN2", target_bir_lowering=False, debug=False,
                   num_devices=NCORES)

    xT = nc.dram_tensor("xT", [KT, 128, TOK], F16, kind="ExternalInput").ap()
    w = nc.dram_tensor("w", [CH, KT, 128, 4, CW], F16, kind="ExternalInput").ap()
    boz = nc.dram_tensor("boz", [2, CH, CW], F16, kind="ExternalInput").ap()
    # packed elementwise inputs: [tok, chunk, (mi|ci|ni|ho), cw] - 4KiB runs
    elin = nc.dram_tensor("elin", [TOK, CH, 4, CW], F16,
                          kind="ExternalInput").ap()
    mpk = nc.dram_tensor("mpk", [NT, 128, 3], F32, kind="ExternalInput").ap()

    # packed outputs: [tok, chunk, (mt|nt|ct|ht), cw]
    out = nc.dram_tensor("out", [TOK, CH, 4, CW], F16,
                         kind="ExternalOutput").ap()

    with tile.TileContext(nc) as tc:
        with (
            tc.tile_pool(name="singles", bufs=1) as singles,
            tc.tile_pool(name="inpool", bufs=3) as inpool,
            tc.tile_pool(name="tmpA", bufs=2) as tmpA,
            tc.tile_pool(name="tmpA1", bufs=1) as tmpA1,
            tc.tile_pool(name="tmpB", bufs=2) as tmpB,
            tc.tile_pool(name="outp", bufs=3) as outp,
            tc.tile_pool(name="ps", bufs=8, space="PSUM") as pspool,
        ):
            # All weights stay resident in SBUF (18 MiB total with xT):
            # no per-iteration weight reload. Interleave xT k-tiles with the
            # first chunk's weight tiles so the first matmul starts after
            # ~1 MiB of DMA; chunk-1 weights queue behind (needed ~170us in).
            xk_sb = []
            wk_res = [[None] * KT for _ in range(CH)]
            for k in range(KT):
                xk = singles.tile([128, TOK], F16, name=f"xT{k}")
                nc.sync.dma_start(out=xk, in_=xT[k])
                xk_sb.append(xk)
                wt = singles.tile([128, 4, CW], F16, name=f"w0_{k}")
                nc.sync.dma_start(out=wt, in_=w[0, k])
                wk_res[0][k] = wt
            for k in range(KT):
                wt = singles.tile([128, 4, CW], F16, name=f"w1_{k}")
                nc.sync.dma_start(out=wt, in_=w[1, k])
                wk_res[1][k] = wt
            # small control tensors go on the gpsimd queue: available almost
            # immediately, never queued behind the bulk preamble
            mpk_sb = singles.tile([128, NT, 3], F32)
            nc.gpsimd.dma_start(out=mpk_sb,
                                in_=mpk.rearrange("t p c -> p t c"))
            # partition-broadcast O/Z bias tiles [128, CH, CW]
            bb_sb = singles.tile([128, 2, CH, CW], F16)
            for gi in range(2):
                for cj in range(CH):
                    bsl = boz[gi, cj]
                    bcast = bass.AP(tensor=bsl.tensor, offset=bsl.offset,
                                    ap=[[0, 128]] + list(bsl.ap))
                    nc.gpsimd.dma_start(out=bb_sb[:, gi, cj], in_=bcast)

            for _ in range(repeat):
                for c in range(CH):
                    wk = wk_res[c]
                    for t in range(NT):
                        rows = ts(t, 128)
                        ein = inpool.tile([128, 4, CW], F16, tag="ein")
                        nc.gpsimd.dma_start(out=ein, in_=elin[rows, c])
                        mi_t, ci_t, ni_t, ho_t = (ein[:, 0], ein[:, 1],
                                                  ein[:, 2], ein[:, 3])
                        m_ap = mpk_sb[:, t, 0:1]
                        om_ap = mpk_sb[:, t, 1:2]
                        hm_ap = mpk_sb[:, t, 2:3]

                        ps = [pspool.tile([128, CW], F32, tag="ps",
                                          name=f"ps{g}") for g in range(4)]
                        for k in range(KT):
                            lhsT = xk_sb[k][:, rows]
                            for g in range(4):
                                nc.tensor.matmul(ps[g], lhsT, wk[k][:, g],
                                                 start=(k == 0),
                                                 stop=(k == KT - 1))

                        # engine streams are FIFO: issue order within each
                        # engine is chosen by operand availability so no
                        # stream head-blocks. The N/rec path (longest) gets
                        # priority; FN/p1/t2 run on the otherwise-idle GpSimd.
                        psI, psF, psO, psZ = ps
                        tmpI = tmpB.tile([128, CW], F16, tag="tmpI")
                        nc.scalar.activation(tmpI, psI, AF.Copy)
                        A = tmpA.tile([128, CW], F16, tag="A")
                        nc.vector.tensor_add(A, psF, mi_t)
                        Dd = tmpA.tile([128, CW], F16, tag="Dd")
                        nc.vector.tensor_sub(Dd, A, tmpI)
                        p_ = tmpA.tile([128, CW], F16, tag="p")
                        nc.vector.tensor_scalar_min(p_, Dd, 0.0)
                        pn = tmpA.tile([128, CW], F16, tag="pn")
                        nc.vector.tensor_scalar(pn, Dd, -1.0, 0.0, OP.mult,
                                                OP.min)
                        Ft = tmpB.tile([128, CW], F16, tag="Ft")
                        nc.scalar.activation(Ft, p_, AF.Exp)
                        It = tmpB.tile([128, CW], F16, tag="It")
                        nc.scalar.activation(It, pn, AF.Exp)

                        eoutMN = outp.tile([128, 2, CW], F16, tag="eoutMN")
                        Mt, Nt = eoutMN[:, 0], eoutMN[:, 1]
                        eoutCH = outp.tile([128, 2, CW], F16, tag="eoutCH")
                        Ct, Ht = eoutCH[:, 0], eoutCH[:, 1]

                        nc.vector.tensor_sub(Mt, A, p_)
                        mF = tmpA1.tile([128, CW], F16, tag="mF")
                        nc.vector.tensor_scalar(mF, Ft, m_ap, om_ap, OP.mult,
                                                OP.add)
                        tO = tmpA.tile([128, CW], F16, tag="tO")
                        nc.vector.tensor_add(tO, psO, bb_sb[:, 0, c])
                        th = tmpB.tile([128, CW], F16, tag="th")
                        nc.scalar.activation(th, tO, AF.Tanh, scale=0.5)
                        tZ = tmpA.tile([128, CW], F16, tag="tZ")
                        nc.vector.tensor_add(tZ, psZ, bb_sb[:, 1, c])
                        Zt = tmpB.tile([128, CW], F16, tag="Zt")
                        nc.scalar.activation(Zt, tZ, AF.Tanh)

                        FN = tmpA1.tile([128, CW], F16, tag="FN")
                        nc.gpsimd.tensor_mul(FN, Ft, ni_t)
                        p1 = tmpA1.tile([128, CW], F16, tag="p1")
                        nc.gpsimd.tensor_mul(p1, ci_t, mF)
                        NtF = tmpA1.tile([128, CW], F32, tag="NtF")
                        nc.vector.tensor_add(NtF, FN, It)
                        nc.scalar.activation(Nt, NtF, AF.Copy)
                        rec = tmpB.tile([128, CW], F32, tag="rec")
                        nc.vector.reciprocal_approx_fast(rec, NtF)
                        nc.sync.dma_start(out=out[rows, c, 0:2], in_=eoutMN)

                        t2 = tmpA1.tile([128, CW], F16, tag="t2")
                        nc.gpsimd.tensor_mul(t2, Zt, It)
                        nc.vector.scalar_tensor_tensor(Ct, t2, m_ap, p1,
                                                       OP.mult, OP.add)

                        thp = tmpA1.tile([128, CW], F16, tag="thp")
                        nc.vector.tensor_scalar(thp, th, hm_ap, hm_ap,
                                                OP.mult, OP.add)
                        x1 = tmpA1.tile([128, CW], F16, tag="x1")
                        nc.vector.tensor_mul(x1, Ct, rec)
                        x2 = tmpA1.tile([128, CW], F16, tag="x2")
                        nc.vector.tensor_mul(x2, x1, thp)
                        nc.vector.tensor_add(Ht, x2, ho_t)

                        nc.sync.dma_start(out=out[rows, c, 2:4], in_=eoutCH)

    nc.compile()
    _CACHE[repeat] = nc
    return nc


def _prep_inputs(inputs):
    f32, f16 = np.float32, np.float16
    g = {k: np.asarray(v) for k, v in inputs.items()}

    Wh = np.concatenate([g['WI_w'], g['WF_w'], g['WO_w'], g['WZ_w']], axis=1)
    Wz = np.concatenate([g['RI_w'], g['RF_w'], g['RO_w'], g['RZ_w']], axis=1)
    bias = np.concatenate([g['WI_b'] + g['RI_b'], g['WF_b'] + g['RF_b'],
                           g['WO_b'] + g['RO_b'], g['WZ_b'] + g['RZ_b']])
    Wcat = np.vstack([Wh, Wz]).astype(f16)                   # [1536, 4096]
    w_l = np.ascontiguousarray(
        Wcat.reshape(KT, 128, 4, CH, CW).transpose(3, 0, 1, 2, 4))
    bI, bF, bO, bZ = bias.reshape(4, DFF).astype(f32)
    boz_l = np.ascontiguousarray(
        np.stack([bO, bZ]).reshape(2, CH, CW).astype(f16))
    mi_shift = (bF - bI)[None, :]

    in_maps = []
    for c in range(NCORES):
        sl = slice(c * BL, (c + 1) * BL)
        Hi_c = g['Hi'][sl].reshape(TOK, DFF)
        Zi_c = g['Zi'][sl].reshape(TOK, D)
        m_c = g['m'][sl].reshape(TOK, 1).astype(f32)
        X = np.concatenate([Hi_c, Zi_c], axis=1)
        xT = np.ascontiguousarray(X.T).astype(f16).reshape(KT, 128, TOK)
        mpk = np.concatenate([m_c, 1.0 - m_c, 0.5 * m_c],
                             axis=1).astype(f32).reshape(NT, 128, 3)
        elin = np.empty((TOK, CH, 4, CW), f16)
        for j, arr in enumerate([
                g['Mi'][sl].reshape(TOK, DFF) + mi_shift,
                g['Ci'][sl].reshape(TOK, DFF),
                g['Ni'][sl].reshape(TOK, DFF),
                (1.0 - m_c) * Hi_c]):
            elin[:, :, j, :] = arr.astype(f16).reshape(TOK, CH, CW)
        in_maps.append({
            "xT": xT,
            "w": w_l,
            "boz": boz_l,
            "elin": elin,
            "mpk": mpk,
        })
    return in_maps, bI


def _gather(results, bI):
    def cat(j):
        full = np.concatenate(
            [results[c]["out"][:, :, j, :].astype(np.float32)
             .reshape(BL, P, DFF) for c in range(NCORES)], axis=0)
        return np.ascontiguousarray(full, dtype=np.float32)
    mt = cat(0)
    mt += bI.reshape(1, 1, DFF)
    return cat(2), mt, cat(3), cat(1)


def kernel(**inputs):
    nc = _build(repeat=1)
    in_maps, bI = _prep_inputs(inputs)
    res = bass_utils.run_bass_kernel_spmd(nc, in_maps,
                                          core_ids=list(range(NCORES)))
    return _gather(res.results, bI)

